# revision 1
# baseline (speedup 1.0000x reference)
"""RWKV v5.2 single-token forward on 8 Trainium2 NeuronCores — v2.

Tensor-parallel over heads (2 heads/core).  Host folds layernorm+token-mix
into the weights so device matvecs run on raw x; per-layer LN stats are
computed with a single all-ones matmul (reduce+broadcast in one shot) and
fixed up with two fused DVE ops.  Weights are single fp16 (tolerance is
2e-2; fp16 gives ~4e-4), halving HBM traffic and matmul count vs hi/lo.

Cross-core all-reduce: XOR all-to-all via remote_dma_broadcast; descriptor
generation is hoisted to the start of each phase (data reads defer to the
trigger), so only trigger+drain+wire remain on the critical path.
"""

import numpy as np

import concourse.bass as bass
import concourse.tile as tile
from concourse import bacc, mybir
from concourse.bass_utils import run_bass_kernel_spmd

L, D, H, S, FF = 12, 1024, 16, 64, 3584
NCORES = 8
HL = H // NCORES        # heads per core (2)
RD = D // NCORES        # 128 output rows per core for D-dim shards
RF = FF // NCORES       # 448 ff rows per core
CH = RF // 4            # 112: ff chunk (partition dim of fk psum / fvw lhsT)
NDC = D // 128          # 8 chunks of the D-dim contraction
EPS = 1e-5
dt = mybir.dt.float32
dth = mybir.dt.float16
AX = mybir.AxisListType
OP = mybir.AluOpType
AF = mybir.ActivationFunctionType

# ------------------------------------------------------------ wblob layout
_segs = [
    ("kvrg", 4 * NDC * 128),   # 4 matrices, lhsT [128, 128] per d-chunk
    ("ow", NDC * 128),         # lhsT [128(d), 128(m)] per m-chunk
    ("frw", NDC * 128),
    ("fkw", NDC * 4 * 128),    # lhsT [128(d), 128(m)] per (kc, mc); ff rows 448:512 pad
    ("fvw", 4 * NDC * 128),    # lhsT [128(ff), 128(m)]; ff rows 448:512 pad
    ("sbd", 128),              # block-diag wkv state, lhsT [128(s), 128(d)]
]
_off = {}
_f = 0
for _n, _sz in _segs:
    _off[_n] = _f
    _f += _sz
WB = _f

# cblob: fp32 consts, all layers in one tile; per-layer stride CW
CW = 21
CO = {"ksum4": 0, "kbias4": 4, "fksum4": 8, "fkbias4": 12,
      "frsum": 16, "frbias": 17, "tf": 18, "lxw": 19, "lxb": 20}

# gconst cols
GC_EPS = 0
GC_CVD = 1     # (-1/D, 1/D)
GC_CVS = 3     # (-1/S, 1/S)
GC_MASK = 5    # 8 cols, one-hot my-core
GC_W = 13


def _ap3(ap, c):
    return ap.rearrange("(p c) -> p c", c=c)


# ---------------------------------------------------------------- device build
def _build_nc():
    nc = bacc.Bacc("TRN2", target_bir_lowering=False, debug=False,
                   num_devices=NCORES, num_swdge_queues=2)

    blob_in = nc.dram_tensor("blob", [L, 128, WB], dth, kind="ExternalInput").ap()
    cb_in = nc.dram_tensor("cblob", [128, L * CW], dt, kind="ExternalInput").ap()
    x0_in = nc.dram_tensor("x0", [128, NDC], dt, kind="ExternalInput").ap()
    gc_in = nc.dram_tensor("gconst", [128, GC_W], dt, kind="ExternalInput").ap()
    mats_in = nc.dram_tensor("gmats", [128, 256], dt, kind="ExternalInput").ap()
    x_out = nc.dram_tensor("x_out", [D], dt, kind="ExternalOutput").ap()

    bar_in = nc.dram_tensor("bar_in", [4], dt)
    bar_out = nc.dram_tensor("bar_out", [NCORES, 4], dt, addr_space="Shared")
    RG = [list(range(NCORES))]

    post_waits = []  # (mybir ins, sem, val) attached after scheduling

    with tile.TileContext(nc) as tc:
        with tc.tile_pool(name="wp", bufs=2) as wp, \
             tc.tile_pool(name="sm", bufs=3) as sm, \
             tc.tile_pool(name="sx", bufs=3) as sx, \
             tc.tile_pool(name="cst", bufs=1) as cst, \
             tc.tile_pool(name="rx", bufs=2) as rx, \
             tc.tile_pool(name="pmv", bufs=2, space="PSUM") as pmv, \
             tc.tile_pool(name="pst", bufs=2, space="PSUM") as pst, \
             tc.tile_pool(name="pwk", bufs=2, space="PSUM") as pwk, \
             tc.tile_pool(name="pbg", bufs=2, space="PSUM") as pbg:

            gc = cst.tile([128, GC_W], dt)
            nc.sync.dma_start(gc[:], gc_in[:])
            mats = cst.tile([128, 256], dt)
            nc.sync.dma_start(mats[:], mats_in[:])
            cb = cst.tile([128, L * CW], dt)
            nc.sync.dma_start(cb[:], cb_in[:])
            AON = mats[:, 0:128]
            BHD = mats[:, 128:256]
            epsc = gc[:, GC_EPS:GC_EPS + 1]
            cvD = gc[:, GC_CVD:GC_CVD + 2]
            cvS = gc[:, GC_CVS:GC_CVS + 2]
            mask8 = gc[:, GC_MASK:GC_MASK + 8]

            si = sx.tile([128, 16], dt, tag="si")
            nc.sync.dma_start(si[:, 0:NDC], x0_in[:])
            x16 = sm.tile([128, NDC], dth, tag="x16")
            nc.vector.tensor_copy(x16[:], si[:, 0:NDC])
            nc.vector.tensor_mul(si[:, 8:16], si[:, 0:8], si[:, 0:8])

            rsems = [nc.alloc_semaphore("rsem_att"),
                     nc.alloc_semaphore("rsem_ffn")]
            lsem = nc.alloc_semaphore("rdma_lsem")
            bar = nc.gpsimd.collective_compute(
                "AllGather", OP.bypass, replica_groups=RG,
                ins=[bar_in.ap().opt()], outs=[bar_out.ap().opt()])
            pid = nc.gpsimd.partition_id()
            exn = [0]

            import os
            EXCHANGE = os.environ.get("RWKV_EXCHANGE", "switch")

            def exchange(pay, n, which, l, name):
                """All-to-all of `pay` [128, n].  Must be called after the
                last write of pay (descs read the data at trigger time, but
                trace order must see the writes first).

                "switch": one 8-dest broadcast (self included) whose receive
                slot is indexed by the SENDER's core id via tc.Switch — the
                whole all-to-all in a single ~66-desc/lane drain.
                "a2a": 7 one-dest XOR broadcasts + local self-slot copy."""
                rt = rx.tile([128, 8 * n], dth, tag="", bufs=1, name=name)
                if EXCHANGE == "switch":
                    rd8 = [(0, k) for k in range(8)]
                    for case in tc.Switch(pid, 8):
                        nc.gpsimd.remote_dma_broadcast(
                            rt[:, case * n:(case + 1) * n], pay,
                            remote_sem=rsems[which], local_sem=lsem,
                            rdests=rd8, queue_num=which)
                    # prune trace-time pending to one prep (only one switch
                    # arm executes at runtime; arms have equal Pool ticks)
                    pend = nc.gpsimd._pending_untriggered_insts[which]
                    nc.gpsimd._pending_untriggered_insts[which] = pend[-1:]
                    thresh = 16
                else:
                    for k in range(1, 8):
                        rd = [None] * 8
                        rd[k] = (0, k)
                        nc.gpsimd.remote_dma_broadcast(
                            rt[:, k * n:(k + 1) * n], pay,
                            remote_sem=rsems[which], local_sem=lsem,
                            rdests=rd, queue_num=which)
                    nc.vector.tensor_copy(rt[:, 0:n], pay)
                    thresh = 14
                trig = nc.gpsimd.trigger_dma(count=None, queue_num=which)
                exn[0] += 1
                if exn[0] == 1:
                    tile.add_dep_helper(trig.ins, bar.ins, sync=True,
                                        reason="startup barrier")

                def attach(red_ins):
                    post_waits.append((red_ins, rsems[which],
                                       thresh * (l + 1)))
                    tile.add_dep_helper(red_ins, trig.ins, sync=True,
                                        reason="exchange recv")
                return rt, attach

            def reduce_slots(rt, n, out_ap, attach):
                r3 = rt[:].rearrange("p (r c) -> p c r", c=n)
                red = nc.vector.tensor_reduce(out_ap, r3, axis=AX.X, op=OP.add)
                attach(red.ins)
                return red

            def stats(si_t, tag):
                """LN stats from si=[x|x^2]: returns (rstd, -m*rstd)."""
                psA = pst.tile([128, 16], dt, tag="ps_stat")
                nc.tensor.matmul(psA[:], AON, si_t[:], start=True, stop=True)
                st = sm.tile([128, 8], dt, tag=tag)
                nc.vector.tensor_reduce(
                    st[:, 0:2], psA[:].rearrange("p (a b) -> p a b", b=8),
                    axis=AX.X, op=OP.add)
                nc.vector.tensor_mul(st[:, 2:4], st[:, 0:2], cvD)  # [nm,-ms]
                nc.vector.scalar_tensor_tensor(
                    st[:, 4:5], st[:, 2:3], st[:, 2:3], st[:, 3:4],
                    op0=OP.mult, op1=OP.add)                       # -var
                nc.scalar.activation(st[:, 5:6], st[:, 4:5], AF.Sqrt,
                                     bias=epsc, scale=-1.0)
                nc.vector.reciprocal_approx_fast(st[:, 6:7], st[:, 5:6])
                nc.vector.tensor_mul(st[:, 7:8], st[:, 2:3], st[:, 6:7])
                return st[:, 6:7], st[:, 7:8]   # rstd, -m*rstd

            for l in range(L):
                blob = wp.tile([128, WB], dth, tag="blob")
                NSPLIT = 2
                bchunk = (WB + NSPLIT - 1) // NSPLIT
                for sp in range(NSPLIT):
                    a, b2 = sp * bchunk, min((sp + 1) * bchunk, WB)
                    nc.sync.dma_start(blob[:, a:b2], blob_in[l][:, a:b2])
                co = l * CW

                def W(name, a, b, p=128):
                    o = _off[name]
                    return blob[0:p, o + a: o + b]

                def C(name, w=1, p=128):
                    o = co + CO[name]
                    return cb[0:p, o: o + w]

                # ---------------- attention ----------------
                payA = sm.tile([128, NDC], dth, tag="payA")
                rstd, bmrs = stats(si, "statA")
                psK = pmv.tile([128, 4], dt, tag="ps_mv")
                for j in range(4):
                    for dc in range(NDC):
                        o = (j * NDC + dc) * 128
                        nc.tensor.matmul(psK[:, j:j + 1],
                                         W("kvrg", o, o + 128),
                                         x16[:, dc:dc + 1],
                                         start=(dc == 0), stop=(dc == NDC - 1))
                fix4 = sm.tile([128, 4], dt, tag="fix4")
                nc.vector.scalar_tensor_tensor(
                    fix4[:], C("ksum4", 4), bmrs, C("kbias4", 4),
                    op0=OP.mult, op1=OP.add)
                kvrg = sm.tile([128, 4], dt, tag="kvrg")
                nc.vector.scalar_tensor_tensor(
                    kvrg[:], psK[:], rstd, fix4[:], op0=OP.mult, op1=OP.add)
                k_, v_, r_, g_ = (kvrg[:, i:i + 1] for i in range(4))

                # wkv = alpha_h * v + r^T S ; alpha = BHD @ (k*r*tf)
                wg = sm.tile([128, 4], dt, tag="wg")
                nc.vector.scalar_tensor_tensor(
                    wg[:, 0:1], k_, r_, C("tf"), op0=OP.mult, op1=OP.mult)
                rhl = sm.tile([128, 1], dth, tag="rhl")
                nc.vector.tensor_copy(rhl[:], r_)
                psW = pwk.tile([128, 4], dt, tag="ps_wkv")
                nc.tensor.matmul(psW[:, 0:1], BHD, wg[:, 0:1],
                                 start=True, stop=True)
                nc.tensor.matmul(psW[:, 1:2], W("sbd", 0, 128), rhl[:],
                                 start=True, stop=True)
                nc.vector.scalar_tensor_tensor(
                    wg[:, 1:2], v_, psW[:, 0:1], psW[:, 1:2],
                    op0=OP.mult, op1=OP.add)                       # wkv
                nc.vector.tensor_mul(wg[:, 2:3], wg[:, 1:2], wg[:, 1:2])
                nc.tensor.matmul(psW[:, 2:4], BHD, wg[:, 1:3],
                                 start=True, stop=True)
                g2 = sm.tile([128, 7], dt, tag="g2")
                nc.vector.tensor_mul(g2[:, 0:2], psW[:, 2:4], cvS)  # [nmg,-msg]
                nc.vector.scalar_tensor_tensor(
                    g2[:, 2:3], g2[:, 0:1], g2[:, 0:1], g2[:, 1:2],
                    op0=OP.mult, op1=OP.add)                        # -var
                nc.scalar.activation(g2[:, 3:4], g2[:, 2:3], AF.Sqrt,
                                     bias=epsc, scale=-1.0)
                # sigmoid(g) here: after the gn Sqrt so the sigmoid table
                # load overlaps the remaining DVE chain
                sg = sm.tile([128, 1], dt, tag="sg")
                nc.scalar.activation(sg[:], g_, AF.Sigmoid)
                nc.vector.reciprocal_approx_fast(g2[:, 4:5], g2[:, 3:4])
                nc.vector.tensor_mul(g2[:, 5:6], g2[:, 4:5], C("lxw"))  # s1
                nc.vector.scalar_tensor_tensor(
                    g2[:, 6:7], g2[:, 0:1], g2[:, 5:6], C("lxb"),
                    op0=OP.mult, op1=OP.add)                        # b1
                gn = sm.tile([128, 2], dt, tag="gn")
                nc.vector.scalar_tensor_tensor(
                    gn[:, 0:1], wg[:, 1:2], g2[:, 5:6], g2[:, 6:7],
                    op0=OP.mult, op1=OP.add)
                nc.vector.scalar_tensor_tensor(
                    gn[:, 1:2], gn[:, 0:1], sg[:], g_,
                    op0=OP.mult, op1=OP.mult)                       # gn*sg*g
                ghl = sm.tile([128, 1], dth, tag="ghl")
                nc.vector.tensor_copy(ghl[:], gn[:, 1:2])

                psO = pbg.tile([128, 8], dt, tag="ps_big")
                for mc in range(NDC):
                    o = mc * 128
                    nc.tensor.matmul(psO[:, mc:mc + 1], W("ow", o, o + 128),
                                     ghl[:], start=True, stop=True)

                nc.vector.tensor_copy(payA[:], psO[:])
                rtA, attach = exchange(payA[:], NDC, 0, l, f"rtA{l}")
                # ---------------- channel mixing ----------------
                payF = sm.tile([128, 16], dth, tag="payF")

                si2 = sx.tile([128, 16], dt, tag="si")
                reduce_slots(rtA, NDC, si2[:, 8:16], attach)
                nc.vector.tensor_add(si2[:, 0:8], si[:, 0:8], si2[:, 8:16])
                x16b = sm.tile([128, NDC], dth, tag="x16")
                nc.vector.tensor_copy(x16b[:], si2[:, 0:8])
                nc.vector.tensor_mul(si2[:, 8:16], si2[:, 0:8], si2[:, 0:8])

                rstd2, bmrs2 = stats(si2, "statF")
                psX = pmv.tile([128, 5], dt, tag="ps_mv")
                for kc in range(NDC):
                    o = kc * 128
                    nc.tensor.matmul(psX[:, 4:5], W("frw", o, o + 128),
                                     x16b[:, kc:kc + 1],
                                     start=(kc == 0), stop=(kc == NDC - 1))
                for mc in range(4):
                    for kc in range(NDC):
                        o = (kc * 4 + mc) * 128
                        nc.tensor.matmul(psX[:, mc:mc + 1],
                                         W("fkw", o, o + 128),
                                         x16b[:, kc:kc + 1],
                                         start=(kc == 0), stop=(kc == NDC - 1))
                frfix = sm.tile([128, 1], dt, tag="frfix")
                nc.vector.scalar_tensor_tensor(
                    frfix[:], C("frsum"), bmrs2, C("frbias"),
                    op0=OP.mult, op1=OP.add)
                rr = sm.tile([128, 1], dt, tag="rr")
                nc.scalar.activation(rr[:], psX[:, 4:5], AF.Sigmoid,
                                     bias=frfix[:], scale=rstd2)
                ffix = sm.tile([128, 4], dt, tag="ffix")
                nc.vector.scalar_tensor_tensor(
                    ffix[:], C("fksum4", 4), bmrs2,
                    C("fkbias4", 4), op0=OP.mult, op1=OP.add)
                fk = sm.tile([128, 4], dt, tag="fk")
                nc.vector.scalar_tensor_tensor(
                    fk[:], psX[:, 0:4], rstd2, ffix[:],
                    op0=OP.mult, op1=OP.add)
                nc.vector.tensor_scalar_max(fk[:], fk[:], 0.0)
                khl = sm.tile([128, 4], dth, tag="khl")
                nc.vector.tensor_mul(khl[:], fk[:], fk[:])

                psV = pbg.tile([128, 8], dt, tag="ps_big")
                for mc in range(NDC):
                    for kc in range(4):
                        o = (kc * NDC + mc) * 128
                        nc.tensor.matmul(psV[:, mc:mc + 1],
                                         W("fvw", o, o + 128),
                                         khl[:, kc:kc + 1],
                                         start=(kc == 0), stop=(kc == 3))

                nc.vector.tensor_copy(payF[:, 0:8], psV[:])
                nc.vector.tensor_scalar(payF[:, 8:16], mask8, rr[:], None,
                                        op0=OP.mult)
                rtF, attach2 = exchange(payF[:], 16, 1, l, f"rtF{l}")

                f16t = sm.tile([128, 16], dt, tag="fred")
                reduce_slots(rtF, 16, f16t[:], attach2)
                si3 = sx.tile([128, 16], dt, tag="si")
                nc.vector.tensor_mul(si3[:, 8:16], f16t[:, 0:8], f16t[:, 8:16])
                nc.vector.tensor_add(si3[:, 0:8], si2[:, 0:8], si3[:, 8:16])
                if l < L - 1:
                    x16 = sm.tile([128, NDC], dth, tag="x16")
                    nc.vector.tensor_copy(x16[:], si3[:, 0:8])
                    nc.vector.tensor_mul(si3[:, 8:16], si3[:, 0:8],
                                         si3[:, 0:8])
                si = si3

            nc.sync.dma_start(_ap3(x_out, NDC), si[:, 0:8])

    for ins, sem, val in post_waits:
        bass.BassInstruction(ins).wait_op(sem, val, "sem-ge", check=False)

    nc.compile()
    return nc


# ---------------------------------------------------------------- host shard
def _make_shards(inputs):
    inp = {k: np.asarray(v) for k, v in inputs.items()}
    tok = int(inp["token"][0])

    e = inp["emb_w"][tok].astype(np.float64)
    m, v = e.mean(), e.var()
    x0 = ((e - m) / np.sqrt(v + EPS) * inp["ln0_w"] + inp["ln0_b"]).astype(np.float32)

    shards = []
    for c in range(NCORES):
        rows = slice(c * RD, (c + 1) * RD)
        frows = slice(c * RF, (c + 1) * RF)
        heads = slice(c * HL, (c + 1) * HL)

        blob = np.zeros((L, 128, WB), dtype=np.float16)
        cblob = np.zeros((128, L * CW), dtype=np.float32)

        def put(l, name, seg, p=128):
            o = _off[name]
            blob[l, 0:p, o:o + seg.shape[1]] = seg

        for l in range(L):
            co = l * CW
            l1w, l1b = inp["ln1_w"][l], inp["ln1_b"][l]
            l2w, l2b = inp["ln2_w"][l], inp["ln2_b"][l]
            s_att, s_ffn = inp["state_att_x"][l], inp["state_ffn_x"][l]

            ksum = np.zeros((128, 4), np.float32)
            kbias = np.zeros((128, 4), np.float32)
            seg = np.zeros((128, 4096), np.float16)
            for j, nm in enumerate(["att_kw", "att_vw", "att_rw", "att_gw"]):
                mix = inp[f"att_time_mix_{nm[4]}"][l]
                Wm = inp[nm][l][rows]
                hi = (Wm * (l1w * mix)[None, :]).astype(np.float16)
                cvec = l1b * mix + s_att * (1.0 - mix)
                ksum[:, j] = hi.astype(np.float32).sum(1)
                kbias[:, j] = Wm @ cvec
                seg[:, j * 1024:(j + 1) * 1024] = (
                    hi.T.reshape(NDC, 128, 128).transpose(1, 0, 2).reshape(128, -1))
            put(l, "kvrg", seg)
            cblob[:, co + CO["ksum4"]:co + CO["ksum4"] + 4] = ksum
            cblob[:, co + CO["kbias4"]:co + CO["kbias4"] + 4] = kbias

            put(l, "ow", inp["att_ow"][l][:, rows].T.astype(np.float16))

            mixr = inp["ffn_time_mix_r"][l]
            Wr = inp["ffn_rw"][l][rows]
            hi = (Wr * (l2w * mixr)[None, :]).astype(np.float16)
            cvr = l2b * mixr + s_ffn * (1.0 - mixr)
            cblob[:, co + CO["frsum"]] = hi.astype(np.float32).sum(1)
            cblob[:, co + CO["frbias"]] = Wr @ cvr
            put(l, "frw",
                hi.T.reshape(NDC, 128, 128).transpose(1, 0, 2).reshape(128, -1))

            mixk = inp["ffn_time_mix_k"][l]
            Wk = np.zeros((512, D), np.float32)
            Wk[0:RF] = inp["ffn_kw"][l][frows]
            hi = (Wk * (l2w * mixk)[None, :]).astype(np.float16)
            cvk = l2b * mixk + s_ffn * (1.0 - mixk)
            cblob[:, co + CO["fksum4"]:co + CO["fksum4"] + 4] = (
                hi.astype(np.float32).sum(1).reshape(4, 128).T)
            cblob[:, co + CO["fkbias4"]:co + CO["fkbias4"] + 4] = (
                (Wk @ cvk).reshape(4, 128).T)
            put(l, "fkw",
                hi.T.reshape(NDC, 128, 4, 128).transpose(1, 0, 2, 3).reshape(128, -1))

            Wv = np.zeros((D, 512), np.float16)
            Wv[:, 0:RF] = inp["ffn_vw"][l][:, frows].astype(np.float16)
            put(l, "fvw",
                Wv.T.reshape(4, 128, NDC, 128).transpose(1, 0, 2, 3).reshape(128, -1))

            Sst = inp["state_wkv"][l, heads]
            bd = np.zeros((128, 128), np.float16)
            bd[0:64, 0:64] = Sst[0].astype(np.float16)
            bd[64:128, 64:128] = Sst[1].astype(np.float16)
            put(l, "sbd", bd)
            cblob[:, co + CO["tf"]] = inp["att_time_first"][l, heads].reshape(128)
            cblob[:, co + CO["lxw"]] = inp["att_lnx_w"][l, rows]
            cblob[:, co + CO["lxb"]] = inp["att_lnx_b"][l, rows]

        gconst = np.zeros((128, GC_W), np.float32)
        gconst[:, GC_EPS] = EPS
        gconst[:, GC_CVD] = -1.0 / D
        gconst[:, GC_CVD + 1] = -1.0 / D    # second col negated: gives -ms
        gconst[:, GC_CVS] = -1.0 / S
        gconst[:, GC_CVS + 1] = -1.0 / S
        gconst[:, GC_MASK + c] = 1.0

        gmats = np.zeros((128, 256), np.float32)
        gmats[:, 0:128] = 1.0                       # AON
        gmats[0:64, 128:192] = 1.0                  # BHD block 0
        gmats[64:128, 192:256] = 1.0                # BHD block 1

        shards.append({
            "blob": blob,
            "cblob": cblob,
            "x0": np.ascontiguousarray(x0.reshape(NDC, 128).T),
            "gconst": gconst,
            "gmats": gmats,
        })
    return shards


_NC_CACHE = []


def get_nc():
    if not _NC_CACHE:
        _NC_CACHE.append(_build_nc())
    return _NC_CACHE[0]


def kernel(**inputs):
    nc = get_nc()
    shards = _make_shards(inputs)
    res = run_bass_kernel_spmd(nc, shards, list(range(NCORES)))
    buf = res.results[0]["x_out"]
    return np.ascontiguousarray(
        buf.reshape(128, NDC).T.reshape(D)).astype(np.float32)



# revision 7
# speedup vs baseline: 3.8723x; 3.8723x over previous
"""RWKV v5.2 single-token forward on 8 Trainium2 NeuronCores — v3.

Tensor-parallel over heads (2 heads/core).  Host folds layernorm+token-mix
into the weights so device matvecs run on raw x; per-layer LN stats are
computed with a single all-ones fp16 matmul (reduce+broadcast in one shot)
and fixed up with two fused DVE ops.  Weights are single fp16 (tolerance is
2e-2; fp16 gives ~4e-4).

Cross-core all-reduce (v4): ONE 8-dest remote_dma_broadcast per exchange
whose receive slot is DynSlice-indexed by the SENDER's partition-id register
— no tc.Switch (v2's per-arm CFG joins forced a ~6us GPSIMD library reload
before every exchange) and a single SWDGE frame per exchange (v3's 7
one-dest frames overflowed the desc ring and serialized on DRAINs).
Desc-gen is hoisted to the start of each phase (the data read defers to the
trigger) and chain-ordered after the previous trigger so ring-entry order
matches trigger order.  The weight-blob DMA is split in two chunks gated on
the two exchange triggers of the previous layer, so its packets drain during
the exchange-wait windows instead of queueing ahead of the tiny exchange
payloads on the shared SDMA engines.
"""

import numpy as np

import concourse.bass as bass
import concourse.tile as tile
from concourse import bacc, mybir
from concourse.bass import DynSlice
from concourse.bass_utils import run_bass_kernel_spmd

L, D, H, S, FF = 12, 1024, 16, 64, 3584
NCORES = 8
HL = H // NCORES        # heads per core (2)
RD = D // NCORES        # 128 output rows per core for D-dim shards
RF = FF // NCORES       # 448 ff rows per core
CH = RF // 4            # 112: ff chunk (partition dim of fk psum / fvw lhsT)
NDC = D // 128          # 8 chunks of the D-dim contraction
EPS = 1e-5
dt = mybir.dt.float32
dth = mybir.dt.float16
AX = mybir.AxisListType
OP = mybir.AluOpType
AF = mybir.ActivationFunctionType

# ------------------------------------------------------------ wblob layout
_segs = [
    ("kvrg", 4 * NDC * 128),   # 4 matrices, lhsT [128, 128] per d-chunk
    ("ow", NDC * 128),         # lhsT [128(d), 128(m)] per m-chunk
    ("frw", NDC * 128),
    ("fkw", NDC * 4 * 128),    # lhsT [128(d), 128(m)] per (kc, mc); ff rows 448:512 pad
    ("fvw", 4 * NDC * 128),    # lhsT [128(ff), 128(m)]; ff rows 448:512 pad
    ("sbd", 128),              # block-diag wkv state, lhsT [128(s), 128(d)]
]
_off = {}
_f = 0
for _n, _sz in _segs:
    _off[_n] = _f
    _f += _sz
WB = _f

# cblob: fp32 consts, all layers in one tile; per-layer stride CW
CW = 21
CO = {"ksum4": 0, "kbias4": 4, "fksum4": 8, "fkbias4": 12,
      "frsum": 16, "frbias": 17, "tf": 18, "lxw": 19, "lxb": 20}

# gconst cols
GC_EPS = 0
GC_CVD = 1     # (-1/D, 1/D)
GC_CVS = 3     # (-1/S, 1/S)
GC_MASK = 5    # 8 cols, one-hot my-core
GC_W = 13


def _ap3(ap, c):
    return ap.rearrange("(p c) -> p c", c=c)


# ---------------------------------------------------------------- device build
def _build_nc():
    nc = bacc.Bacc("TRN2", target_bir_lowering=False, debug=False,
                   num_devices=NCORES, num_swdge_queues=2)

    blob_in = nc.dram_tensor("blob", [L, 128, WB], dth, kind="ExternalInput").ap()
    cb_in = nc.dram_tensor("cblob", [128, L * CW], dt, kind="ExternalInput").ap()
    x0_in = nc.dram_tensor("x0", [128, NDC], dt, kind="ExternalInput").ap()
    gc_in = nc.dram_tensor("gconst", [128, GC_W], dt, kind="ExternalInput").ap()
    mats_in = nc.dram_tensor("gmats", [128, 256], dth, kind="ExternalInput").ap()
    x_out = nc.dram_tensor("x_out", [D], dt, kind="ExternalOutput").ap()

    bar_in = nc.dram_tensor("bar_in", [4], dt)
    bar_out = nc.dram_tensor("bar_out", [NCORES, 4], dt, addr_space="Shared")
    RG = [list(range(NCORES))]

    post_waits = []  # (mybir ins, sem, val) attached after scheduling

    with tile.TileContext(nc) as tc:
        with tc.tile_pool(name="wp", bufs=2) as wp, \
             tc.tile_pool(name="sm", bufs=3) as sm, \
             tc.tile_pool(name="sx", bufs=3) as sx, \
             tc.tile_pool(name="cst", bufs=1) as cst, \
             tc.tile_pool(name="rx", bufs=2) as rx, \
             tc.tile_pool(name="pmv", bufs=2, space="PSUM") as pmv, \
             tc.tile_pool(name="pst", bufs=2, space="PSUM") as pst, \
             tc.tile_pool(name="pwk", bufs=2, space="PSUM") as pwk, \
             tc.tile_pool(name="pbg", bufs=2, space="PSUM") as pbg:

            gc = cst.tile([128, GC_W], dt)
            nc.sync.dma_start(gc[:], gc_in[:])
            mats = cst.tile([128, 256], dth)
            nc.sync.dma_start(mats[:], mats_in[:])
            cb = cst.tile([128, L * CW], dt)
            nc.sync.dma_start(cb[:], cb_in[:])
            AON = mats[:, 0:128]
            BHD = mats[:, 128:256]
            epsc = gc[:, GC_EPS:GC_EPS + 1]
            cvD = gc[:, GC_CVD:GC_CVD + 2]
            cvS = gc[:, GC_CVS:GC_CVS + 2]
            mask8 = gc[:, GC_MASK:GC_MASK + 8]

            si = sx.tile([128, 16], dt, tag="si")
            nc.sync.dma_start(si[:, 0:NDC], x0_in[:])
            st16 = sm.tile([128, 16], dth, tag="x16", name="st16_0")
            nc.vector.tensor_copy(st16[:, 0:8], si[:, 0:8])
            nc.vector.tensor_mul(st16[:, 8:16], st16[:, 0:8], st16[:, 0:8])

            rsems = [nc.alloc_semaphore("rsem_att"),
                     nc.alloc_semaphore("rsem_ffn")]
            lsem = nc.alloc_semaphore("rdma_lsem")
            bar = nc.gpsimd.collective_compute(
                "AllGather", OP.bypass, replica_groups=RG,
                ins=[bar_in.ap().opt()], outs=[bar_out.ap().opt()])
            pid8 = nc.gpsimd.partition_id()
            rd8 = [(0, k) for k in range(8)]
            exn = [0]
            prev_trig = [None]

            def exchange_prep(pay, n, which, l, name):
                """One 8-dest broadcast; the receive slot is indexed by the
                SENDER's partition-id register (DynSlice), so every receiver
                gets slot s = sender s's payload.  Descs read `pay` only at
                trigger time, so this can be called before `pay` is written.
                Chain-ordered after the previous trigger so SWDGE ring-entry
                order matches trigger order."""
                rt = rx.tile([128, 8 * n], dth,
                             tag=f"rt{which}", bufs=2, name=name)
                prep = nc.gpsimd.remote_dma_broadcast(
                    rt[:, DynSlice(pid8 * n, n)], pay,
                    remote_sem=rsems[which], local_sem=lsem,
                    rdests=rd8, queue_num=which)
                if prev_trig[0] is not None:
                    tile.add_dep_helper(prep.ins, prev_trig[0].ins, sync=True,
                                        reason="ring order")

                def fire(last_writes):
                    trig = nc.gpsimd.trigger_dma(count=None, queue_num=which)
                    for lw in last_writes:
                        tile.add_dep_helper(trig.ins, lw.ins, sync=True,
                                            reason="payload before trigger")
                    prev_trig[0] = trig
                    exn[0] += 1
                    if exn[0] == 1:
                        tile.add_dep_helper(trig.ins, bar.ins, sync=True,
                                            reason="startup barrier")

                    def attach(red_ins):
                        post_waits.append((red_ins, rsems[which],
                                           16 * (l + 1)))
                        tile.add_dep_helper(red_ins, trig.ins, sync=True,
                                            reason="exchange recv")
                    return trig, attach
                return rt, fire

            def reduce_slots(rt, n, out_ap, attach):
                r3 = rt[:].rearrange("p (r c) -> p c r", c=n)
                red = nc.vector.tensor_reduce(out_ap, r3, axis=AX.X, op=OP.add)
                attach(red.ins)
                return red

            def stats(st_t, tag):
                """LN stats from st16=[x16|x16^2]: returns (rstd, -m*rstd)."""
                psA = pst.tile([128, 16], dt, tag="ps_stat")
                nc.tensor.matmul(psA[:], AON, st_t[:], start=True, stop=True)
                st = sm.tile([128, 8], dt, tag=tag)
                nc.vector.tensor_reduce(
                    st[:, 0:2], psA[:].rearrange("p (a b) -> p a b", b=8),
                    axis=AX.X, op=OP.add)
                nc.vector.tensor_mul(st[:, 2:4], st[:, 0:2], cvD)  # [nm,-ms]
                nc.vector.scalar_tensor_tensor(
                    st[:, 4:5], st[:, 2:3], st[:, 2:3], st[:, 3:4],
                    op0=OP.mult, op1=OP.add)                       # -var
                nc.scalar.activation(st[:, 5:6], st[:, 4:5], AF.Sqrt,
                                     bias=epsc, scale=-1.0)
                nc.vector.reciprocal_approx_fast(st[:, 6:7], st[:, 5:6])
                nc.vector.tensor_mul(st[:, 7:8], st[:, 2:3], st[:, 6:7])
                return st[:, 6:7], st[:, 7:8]   # rstd, -m*rstd

            # layer-0 blob loads immediately (both chunks, ungated)
            NSPLIT = 2
            bchunk = (WB + NSPLIT - 1) // NSPLIT
            blob = wp.tile([128, WB], dth, tag="blob", name="blob_0")
            for sp in range(NSPLIT):
                a, b2 = sp * bchunk, min((sp + 1) * bchunk, WB)
                nc.sync.dma_start(blob[:, a:b2], blob_in[0][:, a:b2])

            for l in range(L):
                co = l * CW

                def W(name, a, b, p=128):
                    o = _off[name]
                    return blob[0:p, o + a: o + b]

                def C(name, w=1, p=128):
                    o = co + CO[name]
                    return cb[0:p, o: o + w]

                # ---------------- attention ----------------
                payA = sm.tile([128, NDC], dth, tag="payA")
                rtA, fireA = exchange_prep(payA[:], NDC, 0, l, f"rtA{l}")

                rstd, bmrs = stats(st16, "statA")
                psK = pmv.tile([128, 4], dt, tag="ps_mv")
                for j in range(4):
                    for dc in range(NDC):
                        o = (j * NDC + dc) * 128
                        nc.tensor.matmul(psK[:, j:j + 1],
                                         W("kvrg", o, o + 128),
                                         st16[:, dc:dc + 1],
                                         start=(dc == 0), stop=(dc == NDC - 1))
                fix4 = sm.tile([128, 4], dt, tag="fix4")
                nc.vector.scalar_tensor_tensor(
                    fix4[:], C("ksum4", 4), bmrs, C("kbias4", 4),
                    op0=OP.mult, op1=OP.add)
                kvrg = sm.tile([128, 4], dt, tag="kvrg")
                nc.vector.scalar_tensor_tensor(
                    kvrg[:], psK[:], rstd, fix4[:], op0=OP.mult, op1=OP.add)
                k_, v_, r_, g_ = (kvrg[:, i:i + 1] for i in range(4))

                # wkv = alpha_h * v + r^T S ; alpha = BHD @ (k*r*tf)
                wgh = sm.tile([128, 1], dth, tag="wgh")
                nc.vector.scalar_tensor_tensor(
                    wgh[:], k_, r_, C("tf"), op0=OP.mult, op1=OP.mult)
                rhl = sm.tile([128, 1], dth, tag="rhl")
                nc.vector.tensor_copy(rhl[:], r_)
                psW = pwk.tile([128, 4], dt, tag="ps_wkv")
                nc.tensor.matmul(psW[:, 0:1], BHD, wgh[:],
                                 start=True, stop=True)
                nc.tensor.matmul(psW[:, 1:2], W("sbd", 0, 128), rhl[:],
                                 start=True, stop=True)
                wkv_t = sm.tile([128, 1], dt, tag="wkv")
                nc.vector.scalar_tensor_tensor(
                    wkv_t[:], v_, psW[:, 0:1], psW[:, 1:2],
                    op0=OP.mult, op1=OP.add)                       # wkv
                w16 = sm.tile([128, 2], dth, tag="w16")
                nc.vector.tensor_copy(w16[:, 0:1], wkv_t[:])
                nc.vector.tensor_mul(w16[:, 1:2], wkv_t[:], wkv_t[:])
                nc.tensor.matmul(psW[:, 2:4], BHD, w16[:],
                                 start=True, stop=True)
                g2 = sm.tile([128, 7], dt, tag="g2")
                nc.vector.tensor_mul(g2[:, 0:2], psW[:, 2:4], cvS)  # [nmg,-msg]
                nc.vector.scalar_tensor_tensor(
                    g2[:, 2:3], g2[:, 0:1], g2[:, 0:1], g2[:, 1:2],
                    op0=OP.mult, op1=OP.add)                        # -var
                nc.scalar.activation(g2[:, 3:4], g2[:, 2:3], AF.Sqrt,
                                     bias=epsc, scale=-1.0)
                # sigmoid(g) here: after the gn Sqrt so the sigmoid table
                # load overlaps the remaining DVE chain
                sg = sm.tile([128, 1], dt, tag="sg")
                nc.scalar.activation(sg[:], g_, AF.Sigmoid)
                nc.vector.reciprocal_approx_fast(g2[:, 4:5], g2[:, 3:4])
                nc.vector.tensor_mul(g2[:, 5:6], g2[:, 4:5], C("lxw"))  # s1
                nc.vector.scalar_tensor_tensor(
                    g2[:, 6:7], g2[:, 0:1], g2[:, 5:6], C("lxb"),
                    op0=OP.mult, op1=OP.add)                        # b1
                gn = sm.tile([128, 2], dt, tag="gn")
                nc.vector.scalar_tensor_tensor(
                    gn[:, 0:1], wkv_t[:], g2[:, 5:6], g2[:, 6:7],
                    op0=OP.mult, op1=OP.add)
                nc.vector.scalar_tensor_tensor(
                    gn[:, 1:2], gn[:, 0:1], sg[:], g_,
                    op0=OP.mult, op1=OP.mult)                       # gn*sg*g
                ghl = sm.tile([128, 1], dth, tag="ghl")
                nc.vector.tensor_copy(ghl[:], gn[:, 1:2])

                psO = pbg.tile([128, 8], dt, tag="ps_big")
                for mc in range(NDC):
                    o = mc * 128
                    nc.tensor.matmul(psO[:, mc:mc + 1], W("ow", o, o + 128),
                                     ghl[:], start=True, stop=True)

                cpA = nc.vector.tensor_copy(payA[:], psO[:])
                trigA, attachA = fireA([cpA])

                if l + 1 < L:
                    nblob = wp.tile([128, WB], dth, tag="blob",
                                    name=f"blob_{l + 1}")
                    a, b2 = 0, bchunk
                    d0 = nc.sync.dma_start(nblob[:, a:b2],
                                           blob_in[l + 1][:, a:b2])
                    tile.add_dep_helper(d0.ins, trigA.ins, sync=True,
                                        reason="blob c0 after trigA")

                # ---------------- channel mixing ----------------
                payF = sm.tile([128, 16], dth, tag="payF")
                rtF, fireF = exchange_prep(payF[:], 16, 1, l, f"rtF{l}")

                si2 = sx.tile([128, 16], dt, tag="si")
                reduce_slots(rtA, NDC, si2[:, 8:16], attachA)
                nc.vector.tensor_add(si2[:, 0:8], si[:, 0:8], si2[:, 8:16])
                st16b = sm.tile([128, 16], dth, tag="x16")
                nc.vector.tensor_copy(st16b[:, 0:8], si2[:, 0:8])
                nc.vector.tensor_mul(st16b[:, 8:16], st16b[:, 0:8],
                                     st16b[:, 0:8])

                rstd2, bmrs2 = stats(st16b, "statF")
                psX = pmv.tile([128, 5], dt, tag="ps_mv")
                for kc in range(NDC):
                    o = kc * 128
                    nc.tensor.matmul(psX[:, 4:5], W("frw", o, o + 128),
                                     st16b[:, kc:kc + 1],
                                     start=(kc == 0), stop=(kc == NDC - 1))
                for mc in range(4):
                    for kc in range(NDC):
                        o = (kc * 4 + mc) * 128
                        nc.tensor.matmul(psX[:, mc:mc + 1],
                                         W("fkw", o, o + 128),
                                         st16b[:, kc:kc + 1],
                                         start=(kc == 0), stop=(kc == NDC - 1))
                frfix = sm.tile([128, 1], dt, tag="frfix")
                nc.vector.scalar_tensor_tensor(
                    frfix[:], C("frsum"), bmrs2, C("frbias"),
                    op0=OP.mult, op1=OP.add)
                rr = sm.tile([128, 1], dt, tag="rr")
                nc.scalar.activation(rr[:], psX[:, 4:5], AF.Sigmoid,
                                     bias=frfix[:], scale=rstd2)
                ffix = sm.tile([128, 4], dt, tag="ffix")
                nc.vector.scalar_tensor_tensor(
                    ffix[:], C("fksum4", 4), bmrs2,
                    C("fkbias4", 4), op0=OP.mult, op1=OP.add)
                fk = sm.tile([128, 4], dt, tag="fk")
                nc.vector.scalar_tensor_tensor(
                    fk[:], psX[:, 0:4], rstd2, ffix[:],
                    op0=OP.mult, op1=OP.add)
                nc.vector.tensor_scalar_max(fk[:], fk[:], 0.0)
                khl = sm.tile([128, 4], dth, tag="khl")
                nc.vector.tensor_mul(khl[:], fk[:], fk[:])

                psV = pbg.tile([128, 8], dt, tag="ps_big")
                for mc in range(NDC):
                    for kc in range(4):
                        o = (kc * NDC + mc) * 128
                        nc.tensor.matmul(psV[:, mc:mc + 1],
                                         W("fvw", o, o + 128),
                                         khl[:, kc:kc + 1],
                                         start=(kc == 0), stop=(kc == 3))

                cpF0 = nc.vector.tensor_copy(payF[:, 0:8], psV[:])
                cpF1 = nc.vector.tensor_scalar(payF[:, 8:16], mask8, rr[:],
                                               None, op0=OP.mult)
                trigF, attachF = fireF([cpF0, cpF1])

                if l + 1 < L:
                    a, b2 = bchunk, WB
                    d1 = nc.sync.dma_start(nblob[:, a:b2],
                                           blob_in[l + 1][:, a:b2])
                    tile.add_dep_helper(d1.ins, trigF.ins, sync=True,
                                        reason="blob c1 after trigF")

                f16t = sm.tile([128, 16], dt, tag="fred")
                reduce_slots(rtF, 16, f16t[:], attachF)
                si3 = sx.tile([128, 16], dt, tag="si")
                nc.vector.tensor_mul(si3[:, 8:16], f16t[:, 0:8], f16t[:, 8:16])
                nc.vector.tensor_add(si3[:, 0:8], si2[:, 0:8], si3[:, 8:16])
                if l < L - 1:
                    st16 = sm.tile([128, 16], dth, tag="x16",
                                   name=f"st16_{l + 1}")
                    nc.vector.tensor_copy(st16[:, 0:8], si3[:, 0:8])
                    nc.vector.tensor_mul(st16[:, 8:16], st16[:, 0:8],
                                         st16[:, 0:8])
                    blob = nblob
                si = si3

            nc.sync.dma_start(_ap3(x_out, NDC), si[:, 0:8])

    for ins, sem, val in post_waits:
        bass.BassInstruction(ins).wait_op(sem, val, "sem-ge", check=False)

    nc.compile()
    return nc


# ---------------------------------------------------------------- host shard
def _make_shards(inputs):
    inp = {k: np.asarray(v) for k, v in inputs.items()}
    tok = int(inp["token"][0])

    e = inp["emb_w"][tok].astype(np.float64)
    m, v = e.mean(), e.var()
    x0 = ((e - m) / np.sqrt(v + EPS) * inp["ln0_w"] + inp["ln0_b"]).astype(np.float32)

    shards = []
    for c in range(NCORES):
        rows = slice(c * RD, (c + 1) * RD)
        frows = slice(c * RF, (c + 1) * RF)
        heads = slice(c * HL, (c + 1) * HL)

        blob = np.zeros((L, 128, WB), dtype=np.float16)
        cblob = np.zeros((128, L * CW), dtype=np.float32)

        def put(l, name, seg, p=128):
            o = _off[name]
            blob[l, 0:p, o:o + seg.shape[1]] = seg

        for l in range(L):
            co = l * CW
            l1w, l1b = inp["ln1_w"][l], inp["ln1_b"][l]
            l2w, l2b = inp["ln2_w"][l], inp["ln2_b"][l]
            s_att, s_ffn = inp["state_att_x"][l], inp["state_ffn_x"][l]

            ksum = np.zeros((128, 4), np.float32)
            kbias = np.zeros((128, 4), np.float32)
            seg = np.zeros((128, 4096), np.float16)
            for j, nm in enumerate(["att_kw", "att_vw", "att_rw", "att_gw"]):
                mix = inp[f"att_time_mix_{nm[4]}"][l]
                Wm = inp[nm][l][rows]
                hi = (Wm * (l1w * mix)[None, :]).astype(np.float16)
                cvec = l1b * mix + s_att * (1.0 - mix)
                ksum[:, j] = hi.astype(np.float32).sum(1)
                kbias[:, j] = Wm @ cvec
                seg[:, j * 1024:(j + 1) * 1024] = (
                    hi.T.reshape(NDC, 128, 128).transpose(1, 0, 2).reshape(128, -1))
            put(l, "kvrg", seg)
            cblob[:, co + CO["ksum4"]:co + CO["ksum4"] + 4] = ksum
            cblob[:, co + CO["kbias4"]:co + CO["kbias4"] + 4] = kbias

            put(l, "ow", inp["att_ow"][l][:, rows].T.astype(np.float16))

            mixr = inp["ffn_time_mix_r"][l]
            Wr = inp["ffn_rw"][l][rows]
            hi = (Wr * (l2w * mixr)[None, :]).astype(np.float16)
            cvr = l2b * mixr + s_ffn * (1.0 - mixr)
            cblob[:, co + CO["frsum"]] = hi.astype(np.float32).sum(1)
            cblob[:, co + CO["frbias"]] = Wr @ cvr
            put(l, "frw",
                hi.T.reshape(NDC, 128, 128).transpose(1, 0, 2).reshape(128, -1))

            mixk = inp["ffn_time_mix_k"][l]
            Wk = np.zeros((512, D), np.float32)
            Wk[0:RF] = inp["ffn_kw"][l][frows]
            hi = (Wk * (l2w * mixk)[None, :]).astype(np.float16)
            cvk = l2b * mixk + s_ffn * (1.0 - mixk)
            cblob[:, co + CO["fksum4"]:co + CO["fksum4"] + 4] = (
                hi.astype(np.float32).sum(1).reshape(4, 128).T)
            cblob[:, co + CO["fkbias4"]:co + CO["fkbias4"] + 4] = (
                (Wk @ cvk).reshape(4, 128).T)
            put(l, "fkw",
                hi.T.reshape(NDC, 128, 4, 128).transpose(1, 0, 2, 3).reshape(128, -1))

            Wv = np.zeros((D, 512), np.float16)
            Wv[:, 0:RF] = inp["ffn_vw"][l][:, frows].astype(np.float16)
            put(l, "fvw",
                Wv.T.reshape(4, 128, NDC, 128).transpose(1, 0, 2, 3).reshape(128, -1))

            Sst = inp["state_wkv"][l, heads]
            bd = np.zeros((128, 128), np.float16)
            bd[0:64, 0:64] = Sst[0].astype(np.float16)
            bd[64:128, 64:128] = Sst[1].astype(np.float16)
            put(l, "sbd", bd)
            cblob[:, co + CO["tf"]] = inp["att_time_first"][l, heads].reshape(128)
            cblob[:, co + CO["lxw"]] = inp["att_lnx_w"][l, rows]
            cblob[:, co + CO["lxb"]] = inp["att_lnx_b"][l, rows]

        gconst = np.zeros((128, GC_W), np.float32)
        gconst[:, GC_EPS] = EPS
        gconst[:, GC_CVD] = -1.0 / D
        gconst[:, GC_CVD + 1] = -1.0 / D    # second col negated: gives -ms
        gconst[:, GC_CVS] = -1.0 / S
        gconst[:, GC_CVS + 1] = -1.0 / S
        gconst[:, GC_MASK + c] = 1.0

        gmats = np.zeros((128, 256), np.float16)
        gmats[:, 0:128] = 1.0                       # AON
        gmats[0:64, 128:192] = 1.0                  # BHD block 0
        gmats[64:128, 192:256] = 1.0                # BHD block 1

        shards.append({
            "blob": blob,
            "cblob": cblob,
            "x0": np.ascontiguousarray(x0.reshape(NDC, 128).T),
            "gconst": gconst,
            "gmats": gmats,
        })
    return shards


_NC_CACHE = []


def get_nc():
    if not _NC_CACHE:
        _NC_CACHE.append(_build_nc())
    return _NC_CACHE[0]


def kernel(**inputs):
    nc = get_nc()
    shards = _make_shards(inputs)
    res = run_bass_kernel_spmd(nc, shards, list(range(NCORES)))
    buf = res.results[0]["x_out"]
    return np.ascontiguousarray(
        buf.reshape(128, NDC).T.reshape(D)).astype(np.float32)
